# revision 14
# baseline (speedup 1.0000x reference)
"""Trainium2 Bass kernel: conditional logistic regression (segmented softmax).

Problem: X [N=4194304, 64] fp32, sorted segment_ids [N] (65536 segments,
avg 64 rows), W [1,64], b [1].
  logits = X @ W.T + b ; out = segmented_softmax(logits, segment_ids)

Strategy (8 cores, data-parallel over N), v3 "PE-matvec" design:
  - Each core owns N/8 = 524288 consecutive rows as 128 spans of S = 4096
    rows (one span per SBUF partition), with +-PAD overlap per span so
    every segment intersecting a span's core rows is fully inside its
    window (PAD >= max segment length). Pad rows are computed redundantly
    and discarded; no cross-partition communication.
  - The matvec runs entirely on PE. The host pre-packs X (fp16) into
    "pair-transposed" tiles: rhs[k = p*64 + d, f] = X[row(2c+p, i0+f), d],
    so a matmul against W packed twice along the 128-contraction yields
    logits for two spans at once. A sliding window into a small zero tile
    (zz) places pair c's logits at PSUM partitions (2c, 2c+1); per
    32-partition block, 16 accumulating matmuls (tile_position col
    offsets 0/32/64/96) build a [128, F] PSUM tile of logits laid out
    [span, position] - exactly what the segmented-softmax scans need.
    No on-chip transposes, no PSUM->SBUF copies, no DVE/GPSIMD matvec.
  - Column groups are mixed-width (448s then 256s) so PAD can be the
    minimal 128 while keeping PE instruction count low.
  - ACT applies exp directly PSUM->SBUF (fp16 E; b dropped: constant
    shift cancels in softmax).
  - Forward masked-sum scan (DVE tensor_tensor_scan, reset at segment
    starts) runs chunked per column group, chained through a tiny carry
    tile, so it overlaps the stream and stays off the tail. evh (segment
    totals at end positions) = s_run - s_run*keep_next, also per group
    (no separate not-end mask stream). Backward propagate scans run per
    staggered output window; out = E * exp(-ln(denom)), fp16.
"""

import numpy as np

import concourse.bass as bass
import concourse.tile as tile
from concourse import mybir
from concourse.alu_op_type import AluOpType

F32 = mybir.dt.float32
F16 = mybir.dt.float16
F8 = mybir.dt.float8e4

# Full problem constants
N_FULL = 4194304
D = 64
N_CORES = 8
SPANS = 128


def _rev(ap):
    """Reverse an AP along its (last) free dim."""
    return ap[:, ::-1]


def _split_multi_waits(nc):
    """Hoist extra sync waits into standalone EventSemaphore instructions.

    Engine compute/DMA instruction encodings only support a single sync-wait
    slot (walrus: "Too many sync wait commands"); standalone waits execute on
    the same engine sequencer in program order, so semantics are unchanged.
    """
    exempt = ()
    n = 0
    for f in nc.m.functions:
        for blk in f.blocks:
            insts = list(blk.instructions)
            out = []
            for ins in insts:
                si = ins.sync_info
                if (
                    si is not None
                    and si.on_wait
                    and len(si.on_wait) > 1
                    and type(ins).__name__ not in exempt
                ):
                    waits = list(si.on_wait)
                    for w in waits[:-1]:
                        es = mybir.InstEventSemaphore(
                            name=f"W-split-{n}", ins=[], outs=[]
                        )
                        n += 1
                        es.engine = ins.engine
                        es.sync_info = mybir.SyncInfo(on_wait=[w], on_update=[])
                        nc.inst_map[es.name] = es
                        out.append(es)
                    ins.sync_info = mybir.SyncInfo(
                        on_wait=[waits[-1]], on_update=list(si.on_update)
                    )
                out.append(ins)
            if len(out) != len(insts):
                blk.instructions = out
    return n


def _choose_layout(S, m):
    """PAD and X-backed column-group widths: minimal PAD >= max(128, m)
    such that L_x = S + PAD (left pad + core; right pads are rebuilt
    on-chip) decomposes into 448s then 256s."""
    p0 = max(128, m)
    for PAD in range(p0, p0 + 4096, 32):
        Lx = S + PAD
        for nb in range(0, Lx // 256 + 1):
            rem = Lx - 256 * nb
            if rem >= 0 and rem % 448 == 0:
                assert PAD <= 512
                return PAD, [448] * (rem // 448) + [256] * nb
    raise ValueError(f"no layout for S={S}, m={m}")


def _make_splits(S, PAD, cums):
    """Output-window split points (core-row coords), staggered so windows
    become ready one column group apart toward the tail."""
    NG = len(cums)
    targets = sorted({max(0, NG - 7), max(0, NG - 5), max(0, NG - 3),
                      max(0, NG - 2)})
    splits = []
    prev = 0
    for g in targets:
        e = cums[g] - 2 * PAD
        if e <= prev + 192 or e > S - 96:
            continue
        splits.append(e)
        prev = e
    splits.append(S)
    return splits


def _chunks(MMG, last):
    """DMA chunk sizes (in matmuls) for one column group."""
    half = max(1, MMG // 2)
    quarter = max(1, MMG // 4)
    if last:
        return [quarter] * (MMG // quarter)
    return [half] * (MMG // half)


def build_nc(S, PAD, groups, spans=SPANS, splits=None, trn=None):
    L = S + 2 * PAD
    Lx = S + PAD
    assert sum(groups) == Lx
    NGX = len(groups)
    NG = NGX + 1                # + pseudo group [Lx, L) built on-chip
    MMG = spans // 2            # span pairs (matmuls) per column group
    QMM = min(16, spans // 2)   # matmuls per DMA chunk
    BLK = min(128, spans)       # partitions per PE column block
    PPB = BLK // 2              # pairs per block = matmuls per DMA chunk
    NBLK = spans // BLK
    cums = [int(c) for c in np.cumsum(groups)] + [L]
    L_tot = spans * S + 2 * PAD
    if splits is None:
        splits = _make_splits(S, PAD, cums)
    assert splits[-1] == S

    XS_X = sum(128 * MMG * F for F in groups)
    XS_TOT = XS_X + 128 * PAD

    nc = bass.Bass(trn, target_bir_lowering=False)
    xs = nc.dram_tensor("xs", [XS_TOT], F16, kind="ExternalInput")
    keepg = nc.dram_tensor("keepg", [L_tot + 1], F8, kind="ExternalInput")
    zz = nc.dram_tensor("zz", [128, 8 * 2 * BLK], F16, kind="ExternalInput")
    out = nc.dram_tensor("out", [spans * S], F16, kind="ExternalOutput")

    # sub-windows: window h covers core rows [e_{h-1}, e_h); its backward
    # scan runs over [PAD + e_{h-1}, min(L, e_h + 2*PAD)).
    sub = []
    prev = 0
    for e in splits:
        b = min(L, e + 2 * PAD)
        ready = next(g for g in range(NG) if cums[g] >= b)
        sub.append((prev, e, b, ready))
        prev = e

    with tile.TileContext(nc) as tc:
        with (
            tc.tile_pool(name="xin", bufs=4) as xin_pool,
            tc.tile_pool(name="pps", bufs=2, space="PSUM") as ppsum_pool,
            tc.tile_pool(name="ot", bufs=2) as ot_pool,
            tc.tile_pool(name="big", bufs=1) as big,
        ):
            zz_sb = big.tile([128, 8 * 2 * BLK], F16, tag="zz")
            nc.scalar.dma_start(out=zz_sb[:, :], in_=zz[:, :])

            keep = big.tile([spans, L + 1], F8, tag="keep")
            E = big.tile([spans, L], F16, tag="E")
            s_run = big.tile([spans, L], F32, tag="srun")
            evh = big.tile([spans, L], F32, tag="evh")
            carry = big.tile([spans, NG], F32, tag="carry")

            nc.scalar.dma_start(
                out=keep[:, :],
                in_=bass.AP(tensor=keepg, offset=0,
                            ap=[[S, spans], [1, L + 1]]),
            )
            nc.vector.memset(keep[:, 0:1], 0.0)
            nc.vector.memset(keep[:, L : L + 1], 0.0)

            def emit_subwindow(h):
                a, e, b, _ = sub[h]
                aw = PAD + a
                w = b - aw
                assert w <= 4095
                nc.vector.tensor_tensor_scan(
                    out=_rev(s_run[:, aw:b]), data0=_rev(keep[:, aw + 1 : b + 1]),
                    data1=_rev(evh[:, aw:b]), initial=0.0,
                    op0=AluOpType.mult, op1=AluOpType.add,
                )
                core = s_run[:, aw : PAD + e]
                # 1/denom as exp(-ln(denom)) on ACT: denom is a positive sum
                # of exps (core rows always hold a full segment total)
                nc.scalar.activation(
                    out=core, in_=core, func=mybir.ActivationFunctionType.Ln,
                )
                nc.scalar.activation(
                    out=core, in_=core, func=mybir.ActivationFunctionType.Exp,
                    scale=-1.0,
                )
                ot = ot_pool.tile([spans, e - a], F16, tag="ot")
                nc.vector.tensor_tensor(
                    out=ot[:, :], in0=E[:, aw : PAD + e], in1=core,
                    op=AluOpType.mult,
                )
                nc.scalar.dma_start(
                    out=bass.AP(tensor=out, offset=a,
                                ap=[[S, spans], [1, e - a]]),
                    in_=ot[:, :],
                )

            emitted = set()

            def emit_scan_group(g, c0, c1):
                # forward masked-sum scan for these columns, chained through
                # the carry tile (bwd scans overwrite s_run, so the chain
                # value is snapshotted right after each chunk)
                nc.vector.tensor_tensor_scan(
                    out=s_run[:, c0:c1], data0=keep[:, c0:c1],
                    data1=E[:, c0:c1],
                    initial=(0.0 if g == 0 else carry[:, g - 1 : g]),
                    op0=AluOpType.mult, op1=AluOpType.add,
                )
                if g < NG - 1:
                    nc.vector.tensor_copy(
                        carry[:, g : g + 1], s_run[:, c1 - 1 : c1]
                    )
                # evh = s_run * (1 - keep_next): segment totals at end rows
                nc.vector.tensor_tensor(
                    out=evh[:, c0:c1], in0=s_run[:, c0:c1],
                    in1=keep[:, c0 + 1 : c1 + 1], op=AluOpType.mult,
                )
                nc.vector.tensor_tensor(
                    out=evh[:, c0:c1], in0=s_run[:, c0:c1], in1=evh[:, c0:c1],
                    op=AluOpType.subtract,
                )
                for h in range(len(sub)):
                    if sub[h][3] == g and h not in emitted:
                        emitted.add(h)
                        emit_subwindow(h)

            # mini group: the very last pair's right-pad columns [Lx, L)
            # (rows past the core's end, incl. the next core's halo). All
            # other spans' right pads are copied from their neighbour's E
            # once group 0 is done.
            xmini = big.tile([128, PAD], F16, tag="xmini")
            nc.sync.dma_start(
                out=xmini[:, :],
                in_=bass.AP(tensor=xs, offset=XS_X, ap=[[PAD, 128], [1, PAD]]),
            )
            cm = MMG - 1
            rm, qm = cm % 8, cm // 8
            Pm = ppsum_pool.tile([spans, PAD], F32, tag="Pm")
            nc.tensor.matmul(
                Pm[:, :],
                lhsT=zz_sb[:, rm * 2 * BLK + BLK - 16 * qm :
                           rm * 2 * BLK + 2 * BLK - 16 * qm],
                rhs=xmini[:, :], start=True, stop=True,
            )
            nc.scalar.activation(
                out=E[:, Lx:L], in_=Pm[:, :],
                func=mybir.ActivationFunctionType.Exp,
            )

            g_copy = next(g for g in range(NGX) if cums[g] >= 2 * PAD)
            xs_off = 0
            for g in range(NGX):
                F = groups[g]
                c1 = cums[g]
                c0 = c1 - F
                P = ppsum_pool.tile([spans, F], F32, tag="P")
                # half-group DMA chunks for stream efficiency (big
                # descriptors); quarters on the last group to keep the
                # post-stream tail short
                cs = _chunks(MMG, last=(g == NGX - 1))
                c = 0
                for qmm in cs:
                    xq = xin_pool.tile([128, qmm * F], F16, tag="xq")
                    nc.sync.dma_start(
                        out=xq[:, :],
                        in_=bass.AP(
                            tensor=xs, offset=xs_off,
                            ap=[[qmm * F, 128], [1, qmm * F]],
                        ),
                    )
                    xs_off += 128 * qmm * F
                    for cc in range(qmm):
                        r, qq = c % 8, c // 8
                        base = r * 2 * BLK + BLK - 16 * qq
                        nc.tensor.matmul(
                            P[:, :],
                            lhsT=zz_sb[:, base : base + BLK],
                            rhs=xq[:, cc * F : (cc + 1) * F],
                            start=(c == 0), stop=(c == MMG - 1),
                        )
                        c += 1
                nc.scalar.activation(
                    out=E[:, c0:c1], in_=P[:, :],
                    func=mybir.ActivationFunctionType.Exp,
                )
                if g == g_copy:
                    # right pads: span q cols [Lx, L) = span q+1 cols
                    # [PAD, 2*PAD) (same global rows). The mini group wrote
                    # the final span; this overwrites spans [0, spans-1).
                    nc.sync.dma_start(
                        out=E[0 : spans - 1, Lx:L],
                        in_=E[1:spans, PAD : 2 * PAD],
                    )
                emit_scan_group(g, c0, c1)
            emit_scan_group(NGX, Lx, L)
            assert len(emitted) == len(sub), (emitted, sub)
    _split_multi_waits(nc)
    return nc


def _prep_host(X, segment_ids, W, S, PAD, groups, spans=SPANS, n_cores=N_CORES):
    N = X.shape[0]
    n_c = spans * S
    assert n_c * n_cores == N
    L = S + 2 * PAD
    Lx = S + PAD
    MMG = spans // 2
    BLK = min(128, spans)

    ids = np.asarray(segment_ids).astype(np.int64)
    idsp = np.concatenate(
        [np.full(PAD, -1, np.int64), ids, np.full(PAD + 1, -1, np.int64)]
    )
    import ml_dtypes
    eq = idsp[1:] == idsp[:-1]
    keep_g = np.concatenate([[False], eq]).astype(ml_dtypes.float8_e4m3)

    Xf = np.asarray(X, dtype=np.float32).astype(np.float16)
    Xp = np.concatenate(
        [np.zeros((PAD, D), np.float16), Xf, np.zeros((PAD, D), np.float16)]
    )
    Wf = np.asarray(W, np.float32).reshape(-1).astype(np.float16)
    zz = np.zeros((128, 8, 2 * BLK), np.float16)
    for r in range(8):
        zz[0:64, r, BLK + 2 * r] = Wf
        zz[64:128, r, BLK + 2 * r + 1] = Wf
    zz = zz.reshape(128, 8 * 2 * BLK)

    st = Xp.strides
    cums = [int(c) for c in np.cumsum(groups)]
    in_maps = []
    for k in range(n_cores):
        lo = k * n_c
        Xc = Xp[lo : lo + n_c + 2 * PAD]
        # V[q, i, d] = row (q*S + i - PAD) of this core's slice
        V = np.lib.stride_tricks.as_strided(
            Xc, shape=(spans, L, D), strides=(S * st[0], st[0], st[1])
        )
        parts = []
        NGX = len(groups)
        for g, F in enumerate(groups):
            c0 = cums[g] - F
            cbase = 0
            for qmm in _chunks(MMG, last=(g == NGX - 1)):
                # [cc, p, i, d] for pairs cbase..cbase+qmm
                blkv = V.reshape(MMG, 2, L, D)[
                    cbase : cbase + qmm, :, c0 : c0 + F, :
                ]
                parts.append(
                    np.ascontiguousarray(
                        blkv.transpose(1, 3, 0, 2)  # [p, d, cc, f]
                    ).reshape(-1)
                )
                cbase += qmm
        # mini tile: last pair, columns [Lx, L)
        xm = V[spans - 2 : spans, Lx:L, :]  # [2, PAD, d]
        parts.append(np.ascontiguousarray(xm.transpose(0, 2, 1)).reshape(-1))
        xs = np.concatenate(parts)
        in_maps.append(
            {
                "xs": xs,
                "keepg": np.ascontiguousarray(keep_g[lo : lo + n_c + 2 * PAD + 1]),
                "zz": zz,
            }
        )
    return in_maps


def _max_seg_len(segment_ids):
    ids = np.asarray(segment_ids).astype(np.int64)
    change = np.flatnonzero(np.diff(ids) != 0)
    starts = np.concatenate([[0], change + 1])
    ends = np.concatenate([change + 1, [len(ids)]])
    return int((ends - starts).max())


def kernel(X, segment_ids, W, b, _return_results=False, _trace=False):
    from concourse import bass_utils

    X = np.asarray(X)
    N = X.shape[0]
    assert N == N_FULL, f"kernel hardcoded for N={N_FULL}, got {N}"
    S = N // (N_CORES * SPANS)
    m = _max_seg_len(segment_ids)
    PAD, groups = _choose_layout(S, m)

    nc = build_nc(S, PAD, groups)
    in_maps = _prep_host(X, segment_ids, W, S, PAD, groups)
    res = bass_utils.run_bass_kernel_spmd(
        nc, in_maps, core_ids=list(range(N_CORES)), trace=_trace
    )
    out = np.concatenate(
        [np.asarray(r["out"], dtype=np.float32) for r in res.results]
    )
    if _return_results:
        return out, res
    return out
